# revision 1
# baseline (speedup 1.0000x reference)
"""GCNConv x2 (shared aggregation) Trainium2 kernel.

out_mu[v]     = sum_{(u,v) in E+self} dinv[u]*dinv[v] * (x[u] @ W1) + b1
out_logstd[v] = same with W2, b2

Key algebraic move: the (linear) neighborhood aggregation commutes with the
dense projection, and both outputs share the same aggregation:
    agg = Dinv (A + I) Dinv X          # [N, 128]
    mu = agg @ W1 + b1 ; logstd = agg @ W2 + b2
so we aggregate once in 128-dim space and run one fused [128, 400] GEMM.

Distribution: nodes (output rows) sharded across 8 cores; edges partitioned by
destination node. x and the weights are replicated so no collectives are
needed; each core gathers the source rows it needs from its own HBM copy.

Device algorithm per core (static SPMD program, fully unrolled):
  - destinations are bin-packed on the host into NB blocks of <=128 "slots"
    with <= T*128 incoming edges (LPT on degree, so blocks are edge-balanced)
  - per block, per 128-edge tile: one indirect-DMA gather (gpsimd, one index
    per partition) pulls 128 source rows (512B each) into SBUF; a one-hot
    indicator M[e, slot] = (iota == slot_e) * norm_e is built with a single
    dual-op tensor_scalar; a matmul per tile accumulates aggT[feat, slot] in
    PSUM; then the fused GEMM (bias added via a rank-1 matmul into the same
    PSUM accumulation group) and a DMA of the [128, 400] result block.
The host then un-permutes block rows to node order (pure indexing).

Why per-128-row indirect gathers: the bulk SWDGE gather (dma_gather /
InstDMAGatherAnt) compiles here but the device-side GPSIMD library is absent
in this environment (device goes unrecoverable), and custom InstISA library
ops (ap_gather etc.) fail walrus codegen. indirect_dma_start honors exactly
one dynamic offset per partition-descriptor, so 128 rows/instruction is the
granularity; measured marginal cost ~1.4us per instruction (Pool-bound).
"""

import numpy as np

N = 100000
C = 128  # in channels
O = 200  # out channels per head
E = 1600000
NCORES = 8
NV = N // NCORES  # dst nodes per core

NB = 100  # blocks per core (NB*128 >= NV slots)
T = 17    # 128-edge gather tiles per block

_programs = {}
last_results = None
TRACE = False


def _build_program(n, c, o, nb, t):
    from contextlib import ExitStack

    import concourse.tile as tile
    from concourse import bass, mybir

    f32 = mybir.dt.float32
    i32 = mybir.dt.int32

    nc = bass.Bass()
    x_h = nc.dram_tensor("x", [n, c], f32, kind="ExternalInput")
    w_h = nc.dram_tensor("w12", [c, 2 * o], f32, kind="ExternalInput")
    b_h = nc.dram_tensor("b12", [1, 2 * o], f32, kind="ExternalInput")
    idx_h = nc.dram_tensor("srcidx", [128, nb * t], i32, kind="ExternalInput")
    met_h = nc.dram_tensor("meta", [nb, 128, 2 * t], f32, kind="ExternalInput")
    out_h = nc.dram_tensor("out", [nb * 128, 2 * o], f32, kind="ExternalOutput")

    iota_np = np.tile(np.arange(128, dtype=np.float32), (128, 1))
    iota_h = nc.inline_tensor(iota_np, "iotac")
    ones_h = nc.inline_tensor(np.ones((1, 128), np.float32), "onesc")

    with ExitStack() as ctx:
        tc = ctx.enter_context(tile.TileContext(nc))
        const = ctx.enter_context(tc.tile_pool(name="const", bufs=1))
        sb = ctx.enter_context(tc.tile_pool(name="sb", bufs=3))
        gpool = ctx.enter_context(tc.tile_pool(name="gpool", bufs=3))
        ps = ctx.enter_context(tc.tile_pool(name="ps", bufs=2, space="PSUM"))
        ops = ctx.enter_context(tc.tile_pool(name="ops", bufs=2, space="PSUM"))

        w12 = const.tile([c, 2 * o], f32)
        nc.sync.dma_start(out=w12[:], in_=w_h[:])
        b12 = const.tile([1, 2 * o], f32)
        nc.sync.dma_start(out=b12[:], in_=b_h[:])
        iota = const.tile([128, 128], f32)
        nc.sync.dma_start(out=iota[:], in_=iota_h[:])
        ones = const.tile([1, 128], f32)
        nc.sync.dma_start(out=ones[:], in_=ones_h[:])
        # all gather indices resident up front: per-tile indirect DMAs then
        # carry only their WAR wait, so the 1-wait walrus limit needs no
        # extra NoOps on the Pool engine
        gidx_all = const.tile([128, nb * t], i32)
        nc.sync.dma_start(out=gidx_all[:], in_=idx_h[:])

        for b in range(nb):
            met = sb.tile([128, 2 * t], f32, tag="met")
            nc.sync.dma_start(out=met[:], in_=met_h[b])

            agg = ps.tile([128, 128], f32, tag="agg")
            # one block-sized gather buffer: the 17 indirect DMAs write
            # disjoint slices, so only the first pays the slot WAR wait and
            # the rest stay within the walrus 1-wait limit (no NoOps)
            gblk = gpool.tile([128, t * 128], f32, tag="g")
            for tt in range(t):
                gs = gblk[:, tt * 128 : (tt + 1) * 128]
                nc.gpsimd.indirect_dma_start(
                    out=gs,
                    out_offset=None,
                    in_=x_h[:],
                    in_offset=bass.IndirectOffsetOnAxis(
                        ap=gidx_all[:, b * t + tt : b * t + tt + 1], axis=0
                    ),
                )
                m = sb.tile([128, 128], f32, tag="m")
                nc.vector.tensor_scalar(
                    out=m[:],
                    in0=iota[:],
                    scalar1=met[:, tt : tt + 1],
                    scalar2=met[:, t + tt : t + tt + 1],
                    op0=mybir.AluOpType.is_equal,
                    op1=mybir.AluOpType.mult,
                )
                nc.tensor.matmul(
                    out=agg[:],
                    lhsT=gblk[:, tt * 128 : (tt + 1) * 128],
                    rhs=m[:],
                    start=(tt == 0),
                    stop=(tt == t - 1),
                )

            aggs = sb.tile([128, 128], f32, tag="aggs")
            nc.any.tensor_copy(out=aggs[:], in_=agg[:])

            op = ops.tile([128, 2 * o], f32, tag="op")
            nc.tensor.matmul(out=op[:], lhsT=ones[:], rhs=b12[:], start=True, stop=False)
            nc.tensor.matmul(out=op[:], lhsT=aggs[:], rhs=w12[:], start=False, stop=True)
            osb = sb.tile([128, 2 * o], f32, tag="osb")
            nc.any.tensor_copy(out=osb[:], in_=op[:])
            nc.sync.dma_start(out=out_h[b * 128 : (b + 1) * 128, :], in_=osb[:])

    return nc


def _split_waits(nc, max_waits=1):
    """Walrus in this toolchain rejects instructions carrying more than
    `max_waits` semaphore waits ("Too many sync wait commands"). Move excess
    waits onto bass_nofuse NoOps inserted just before, on the same engine
    (engines issue in order, so the combined wait condition is preserved)."""
    from concourse import mybir

    for fn in nc.m.functions:
        for bb in fn.blocks:
            new_instrs = []
            changed = False
            for ins in bb.instructions:
                si = getattr(ins, "sync_info", None)
                if si is not None and si.on_wait and len(si.on_wait) > max_waits:
                    waits = list(si.on_wait)
                    keep = waits[-max_waits:]
                    excess = waits[:-max_waits]
                    for i in range(0, len(excess), max_waits):
                        chunk = excess[i : i + max_waits]
                        noop = mybir.InstNoOp(
                            name=nc.get_next_instruction_name(),
                            engine=ins.engine,
                            bass_nofuse=True,
                            sync_info=mybir.SyncInfo(on_wait=chunk, on_update=[]),
                        )
                        new_instrs.append(noop)
                    si.on_wait = keep
                    changed = True
                new_instrs.append(ins)
            if changed:
                bb.instructions[:] = new_instrs


def _get_program(n, c, o, nb, t):
    key = (n, c, o, nb, t)
    if key not in _programs:
        nc = _build_program(n, c, o, nb, t)
        _split_waits(nc)
        _programs[key] = nc
    return _programs[key]


def _pack_core(src_c, dstl_c, norm_c, nv, nb, t):
    """Pack one core's edges (dst-local ids in [0, nv)) into nb blocks of
    <=128 slots and <= t*128 edges. Returns srcidx [nb,128,t] int32,
    meta [nb,128,2t] f32 (slot | norm), rowmap [nb*128] int64 (-1 = pad)."""
    import heapq

    cap = t * 128
    degc = np.bincount(dstl_c, minlength=nv)
    order = np.argsort(-degc, kind="stable")

    block_of = np.empty(nv, np.int32)
    slot_of = np.empty(nv, np.int32)
    slots_used = np.zeros(nb, np.int32)
    load = np.zeros(nb, np.int64)
    heap = [(0, bb) for bb in range(nb)]
    heapq.heapify(heap)
    for v in order:
        d = int(degc[v])
        e, bb = heapq.heappop(heap)
        block_of[v] = bb
        slot_of[v] = slots_used[bb]
        slots_used[bb] += 1
        load[bb] = e + d
        if slots_used[bb] < 128:
            heapq.heappush(heap, (e + d, bb))
    assert load.max() <= cap, (load.max(), cap)

    eb = block_of[dstl_c]
    perm = np.argsort(eb, kind="stable")
    ebs = eb[perm]
    cnt = np.bincount(ebs, minlength=nb)
    starts = np.concatenate([[0], np.cumsum(cnt[:-1])])
    pos = np.arange(len(ebs)) - starts[ebs]
    pp = (pos % 128).astype(np.int64)
    tt = (pos // 128).astype(np.int64)

    srcidx = np.zeros((nb, 128, t), np.int32)
    meta = np.zeros((nb, 128, 2 * t), np.float32)
    srcidx[ebs, pp, tt] = src_c[perm]
    meta[ebs, pp, tt] = slot_of[dstl_c[perm]]
    meta[ebs, pp, t + tt] = norm_c[perm]

    rowmap = np.full(nb * 128, -1, np.int64)
    rowmap[block_of.astype(np.int64) * 128 + slot_of] = np.arange(nv)
    return srcidx, meta, rowmap


def _preprocess(x, edge_index, n, nv, nb, t):
    src = np.asarray(edge_index[0], np.int64)
    dst = np.asarray(edge_index[1], np.int64)
    deg = (np.bincount(dst, minlength=n) + 1).astype(np.float32)
    dinv = 1.0 / np.sqrt(deg)

    loop = np.arange(n, dtype=np.int64)
    src_all = np.concatenate([src, loop])
    dst_all = np.concatenate([dst, loop])
    norm_all = dinv[src_all] * dinv[dst_all]

    per_core = []
    order = np.argsort(dst_all // nv, kind="stable")
    dst_sorted = dst_all[order]
    src_sorted = src_all[order]
    norm_sorted = norm_all[order]
    bounds = np.searchsorted(dst_sorted, np.arange(0, n + 1, nv))
    for cc in range(len(bounds) - 1):
        lo, hi = bounds[cc], bounds[cc + 1]
        per_core.append(
            _pack_core(
                src_sorted[lo:hi],
                (dst_sorted[lo:hi] - cc * nv).astype(np.int64),
                norm_sorted[lo:hi],
                nv,
                nb,
                t,
            )
        )
    return per_core


def kernel(x, edge_index, W1, b1, W2, b2):
    global last_results
    from concourse.bass_utils import run_bass_kernel_spmd

    x = np.ascontiguousarray(np.asarray(x, np.float32))
    W1 = np.asarray(W1, np.float32)
    W2 = np.asarray(W2, np.float32)
    b1 = np.asarray(b1, np.float32)
    b2 = np.asarray(b2, np.float32)
    w12 = np.ascontiguousarray(np.concatenate([W1, W2], axis=1))
    b12 = np.ascontiguousarray(
        np.concatenate([b1, b2]).reshape(1, 2 * O)
    )

    per_core = _preprocess(x, np.asarray(edge_index), N, NV, NB, T)
    nc = _get_program(N, C, O, NB, T)

    in_maps = []
    for cc in range(NCORES):
        srcidx, meta, _rowmap = per_core[cc]
        srcidx_t = np.ascontiguousarray(
            srcidx.transpose(1, 0, 2).reshape(128, NB * T)
        )
        in_maps.append(
            {"x": x, "w12": w12, "b12": b12, "srcidx": srcidx_t, "meta": meta}
        )

    # The axon-tunneled device occasionally comes up wedged from a prior
    # session (NRT_EXEC_UNIT_UNRECOVERABLE on the first execution); the worker
    # restarts on failure, so a retry recovers.
    import time as _time

    res = None
    for attempt in range(3):
        try:
            res = run_bass_kernel_spmd(
                nc, in_maps, list(range(NCORES)), trace=TRACE
            )
            break
        except Exception:
            if attempt == 2:
                raise
            _time.sleep(10.0)
    assert res is not None
    last_results = res

    mu = np.empty((N, O), np.float32)
    logstd = np.empty((N, O), np.float32)
    for cc in range(NCORES):
        _srcidx, _meta, rowmap = per_core[cc]
        dev_out = np.asarray(res.results[cc]["out"])  # [NB*128, 2*O]
        valid = rowmap >= 0
        rows = rowmap[valid] + cc * NV
        mu[rows] = dev_out[valid, :O]
        logstd[rows] = dev_out[valid, O:]
    return mu, logstd

